# revision 10
# baseline (speedup 1.0000x reference)
"""Distributed multi-head attention layer for 8 TRN2 NeuronCores.

Problem: x[2,2048,1024] -> MHA(16 heads, dh=64) -> out[2,2048,1024], f32.

Sharding (per core c in 0..7):
  batch b = c//4, group g = c%4 (4 cores per batch).
  - Each core computes K/V for its 4 heads over the full sequence,
    AllGathers K/V within its 4-core batch group,
  - computes Q for its own 512-query slice over ALL heads,
  - runs attention for all 16 heads x its 512 queries,
  - output-projects to out[b, g*512:(g+1)*512, :]. No output collective.
  Host concatenates per-batch slices and adds the output bias.

Kernel layout choices:
  - x arrives host-transposed (xT [1024,2048]) so projections need no
    on-device transpose.
  - Q/K are produced in [att, s] (transposed) layout; V in natural [s, dh].
  - Scores are computed transposed: S^T[k, q] = KT.T-slice @ QT-slice, so
    softmax's reduction axis (keys) is the partition axis; exp runs on
    ScalarE (PSUM->SBUF), denominators accumulate on VectorE and reduce
    across partitions with a ones-vector matmul. No max-subtraction is
    needed: scores/8 ~ N(0,1), |s| < ~6, exp is safe in f32.
  - All matmuls run as float32r (bit-identical storage to f32, 1 cycle/row
    on the PE at free-dim >= 256 vs 4 cycles/row for plain f32).
  - QK pairs of heads run row-tiled (K=64 each, concurrent); AV pairs run
    column-tiled (M=64 each, concurrent).
"""

import sys

sys.path.insert(0, "/opt/trn_rl_repo")

from contextlib import ExitStack

import ml_dtypes
import numpy as np

import concourse.bass as bass
import concourse.tile as tile
from concourse import bacc, mybir

F32 = mybir.dt.float32
F32R = mybir.dt.float32r
BF16 = mybir.dt.bfloat16
EXP = mybir.ActivationFunctionType.Exp

N_CORES = 8
B, S, D = 2, 2048, 1024
H, DH = 16, 64
QL = 512  # queries per core
GC = 256  # K/V att columns produced per core (4 heads)
KV_KT = 2 * 128 * 2048  # KT part of the kv bounce buffer (elements)
KV_V = 16 * 128 * 256  # V part
KV_N = KV_KT + KV_V
RG = [[0, 1, 2, 3], [4, 5, 6, 7]]

_nc_cache = None


def r(ap):
    return ap


def build_nc():
    nc = bacc.Bacc("TRN2", target_bir_lowering=False, debug=False, num_devices=N_CORES)

    xT_ext = nc.declare_dram_parameter("xT", [D, S], F32R, isOutput=False)
    xq_ext = nc.declare_dram_parameter("xTq", [D, QL], F32R, isOutput=False)
    wq_ext = nc.declare_dram_parameter("wq", [D, D], F32R, isOutput=False)
    wk_ext = nc.declare_dram_parameter("wk", [D, GC], F32R, isOutput=False)
    wv_ext = nc.declare_dram_parameter("wv", [D, GC], F32R, isOutput=False)
    wo_ext = nc.declare_dram_parameter("wo", [D, D], BF16, isOutput=False)
    bq_ext = nc.declare_dram_parameter("bq", [D], F32, isOutput=False)
    bk_ext = nc.declare_dram_parameter("bk", [GC], F32, isOutput=False)
    bv_ext = nc.declare_dram_parameter("bv", [GC], F32R, isOutput=False)
    out_ext = nc.declare_dram_parameter("out", [QL, D], F32, isOutput=True)

    with (
        tile.TileContext(nc) as tc,
        ExitStack() as outer,
        nc.allow_low_precision("float32r is storage-identical to f32; matmul operands require the f32r tag"),
    ):
        # ---- long-lived pools -------------------------------------------
        cpool = outer.enter_context(tc.tile_pool(name="consts", bufs=1))
        qtpool = outer.enter_context(tc.tile_pool(name="qt", bufs=1))
        ocpool = outer.enter_context(tc.tile_pool(name="ocat", bufs=1))
        dramp = outer.enter_context(tc.tile_pool(name="dram", bufs=1, space="DRAM"))
        wop = outer.enter_context(tc.tile_pool(name="wo", bufs=1))

        ones_f32 = cpool.tile([128, 128], F32)
        nc.vector.memset(ones_f32[:], 1.0)
        ones_sb = cpool.tile([128, 128], F32R)
        nc.vector.tensor_copy(ones_sb[:], ones_f32[:])
        ones_bf = cpool.tile([1, 64], BF16)
        nc.vector.tensor_copy(ones_bf[:], ones_f32[0:1, 0:64])
        bq_sb = cpool.tile([128, 8], F32)
        bk_sb = cpool.tile([128, 2], F32)
        bv_sb = cpool.tile([1, GC], F32R)
        for a in range(8):
            nc.sync.dma_start(bq_sb[:, a : a + 1], bq_ext[a * 128 : (a + 1) * 128].unsqueeze(1))
        for a in range(2):
            nc.sync.dma_start(bk_sb[:, a : a + 1], bk_ext[a * 128 : (a + 1) * 128].unsqueeze(1))
        nc.sync.dma_start(bv_sb[:], bv_ext[:].unsqueeze(0))

        QT = [qtpool.tile([128, QL], BF16, name=f"qt{a}") for a in range(8)]
        Ocat = [ocpool.tile([128, QL], BF16, name=f"ocat{i}") for i in range(8)]

        kv_loc = dramp.tile([KV_N], BF16, name="kv_loc")
        kv_gath = dramp.tile([4, KV_N], BF16, name="kv_gath")
        ktl_v = kv_loc[0:KV_KT].rearrange("(t p f) -> t p f", t=2, p=128, f=2048)
        vl_v = kv_loc[KV_KT:KV_N].rearrange("(t p f) -> t p f", t=16, p=128, f=256)

        # ---- phase 1a: K/V projection over full S ------------------------
        with ExitStack() as ph1:
            xtp = ph1.enter_context(tc.tile_pool(name="xt", bufs=1))
            wkvp = ph1.enter_context(tc.tile_pool(name="wkv", bufs=1))
            ktlp = ph1.enter_context(tc.tile_pool(name="ktloc", bufs=1))
            vlp = ph1.enter_context(tc.tile_pool(name="vloc", bufs=1))
            ps1 = ph1.enter_context(tc.tile_pool(name="ps1", bufs=3, space="PSUM"))
            ps1v = ph1.enter_context(tc.tile_pool(name="ps1v", bufs=3, space="PSUM"))

            xT = []
            for kt in range(8):
                t = xtp.tile([128, S], F32R, name=f"xt{kt}")
                nc.sync.dma_start(t[:], xT_ext[kt * 128 : (kt + 1) * 128, :])
                xT.append(t)
            wk_sb, wv_sb = [], []
            for kt in range(8):
                t = wkvp.tile([128, GC], F32R, name=f"wk{kt}")
                nc.sync.dma_start(t[:], wk_ext[kt * 128 : (kt + 1) * 128, :])
                wk_sb.append(t)
                t = wkvp.tile([128, GC], F32R, name=f"wv{kt}")
                nc.sync.dma_start(t[:], wv_ext[kt * 128 : (kt + 1) * 128, :])
                wv_sb.append(t)

            # KT_loc [256(att), 2048(s)] as 2 tiles; evict with bk bias
            KT_loc = [ktlp.tile([128, S], BF16, name=f"ktloc{a}") for a in range(2)]
            for a2 in range(2):
                for sc in range(4):
                    ps = ps1.tile([128, 512], F32, name=f"pskt{a2}_{sc}", tag="ps1")
                    for kt in range(8):
                        nc.tensor.matmul(
                            ps[:],
                            lhsT=r(wk_sb[kt][:, a2 * 128 : (a2 + 1) * 128]),
                            rhs=r(xT[kt][:, sc * 512 : (sc + 1) * 512]),
                            start=(kt == 0),
                            stop=(kt == 7),
                        )
                    nc.vector.tensor_scalar_add(
                        KT_loc[a2][:, sc * 512 : (sc + 1) * 512], ps[:], bk_sb[:, a2 : a2 + 1]
                    )
            # V_loc natural [2048(s), 256(att)] as 16 tiles; bias via ones-matmul
            V_loc = [vlp.tile([128, GC], BF16, name=f"vloc{st}") for st in range(16)]
            for st in range(16):
                ps = ps1v.tile([128, GC], F32, name=f"psv{st}", tag="ps1v")
                for kt in range(8):
                    nc.tensor.matmul(
                        ps[:],
                        lhsT=r(xT[kt][:, st * 128 : (st + 1) * 128]),
                        rhs=r(wv_sb[kt][:]),
                        start=(kt == 0),
                        stop=False,
                    )
                nc.tensor.matmul(
                    ps[:],
                    lhsT=r(ones_sb[0:1, :]),
                    rhs=r(bv_sb[:]),
                    start=False,
                    stop=True,
                )
                nc.vector.tensor_copy(V_loc[st][:], ps[:])

            # bounce K/V to DRAM for the collective
            for a2 in range(2):
                nc.sync.dma_start(ktl_v[a2], KT_loc[a2][:])
            for st in range(16):
                nc.sync.dma_start(vl_v[st], V_loc[st][:])

        # ---- AllGather K/V within each 4-core batch group ----------------
        nc.gpsimd.collective_compute(
            "AllGather",
            mybir.AluOpType.bypass,
            replica_groups=RG,
            ins=[kv_loc.opt()],
            outs=[kv_gath.opt()],
        )

        # ---- phase 1b: Q projection (overlaps the AllGather) -------------
        with ExitStack() as ph1b:
            xqp = ph1b.enter_context(tc.tile_pool(name="xq", bufs=1))
            wqp = ph1b.enter_context(tc.tile_pool(name="wq", bufs=1))
            ps1q = ph1b.enter_context(tc.tile_pool(name="ps1q", bufs=3, space="PSUM"))

            xq_sb, wq_sb = [], []
            for kt in range(8):
                t = xqp.tile([128, QL], F32R, name=f"xq{kt}")
                nc.sync.dma_start(t[:], xq_ext[kt * 128 : (kt + 1) * 128, :])
                xq_sb.append(t)
                t = wqp.tile([128, D], F32R, name=f"wq{kt}")
                nc.sync.dma_start(t[:], wq_ext[kt * 128 : (kt + 1) * 128, :])
                wq_sb.append(t)
            for a in range(8):
                ps = ps1q.tile([128, QL], F32, name=f"psq{a}", tag="ps1q")
                for kt in range(8):
                    nc.tensor.matmul(
                        ps[:],
                        lhsT=r(wq_sb[kt][:, a * 128 : (a + 1) * 128]),
                        rhs=r(xq_sb[kt][:]),
                        start=(kt == 0),
                        stop=(kt == 7),
                    )
                nc.vector.tensor_scalar_add(QT[a][:], ps[:], bq_sb[:, a : a + 1])

        # ---- phase 2: attention over all 16 heads ------------------------
        with ExitStack() as ph2:
            ktgp = ph2.enter_context(tc.tile_pool(name="ktg", bufs=4))
            vgp = ph2.enter_context(tc.tile_pool(name="vg", bufs=32))
            ptp = ph2.enter_context(tc.tile_pool(name="pt", bufs=3))
            dap = ph2.enter_context(tc.tile_pool(name="dacc", bufs=4))
            rcp = ph2.enter_context(tc.tile_pool(name="recip", bufs=2))
            sps = ph2.enter_context(tc.tile_pool(name="sps", bufs=2, space="PSUM"))
            ops = ph2.enter_context(tc.tile_pool(name="ops", bufs=1, space="PSUM"))
            nps = ph2.enter_context(tc.tile_pool(name="nps", bufs=1, space="PSUM"))

            wo_sb = []
            for kt in range(8):
                t = wop.tile([128, D], BF16, name=f"wo{kt}")
                nc.sync.dma_start(t[:], wo_ext[kt * 128 : (kt + 1) * 128, :])
                wo_sb.append(t)

            for gr in range(4):
                gv_kt = kv_gath[gr][0:KV_KT].rearrange("(t p f) -> t p f", t=2, p=128, f=2048)
                gv_v = kv_gath[gr][KV_KT:KV_N].rearrange("(t p f) -> t p f", t=16, p=128, f=256)
                KT_g = []
                for a2 in range(2):
                    t = ktgp.tile([128, S], BF16, name=f"ktg{gr}_{a2}", tag="ktg")
                    nc.sync.dma_start(t[:], gv_kt[a2])
                    KT_g.append(t)
                V_g = []
                for st in range(16):
                    t = vgp.tile([128, GC], BF16, name=f"vg{gr}_{st}", tag="vg")
                    nc.sync.dma_start(t[:], gv_v[st])
                    V_g.append(t)

                for pair in range(2):
                    ktt = KT_g[pair]
                    qtt = QT[gr * 2 + pair]
                    o_ps = ops.tile([128, QL], F32, name=f"ops{gr}_{pair}", tag="ops")
                    dacc = [
                        dap.tile([128, QL], F32R, name=f"da{gr}_{pair}_{w}", tag="dacc")
                        for w in range(2)
                    ]
                    groups = [3] * 10 + [2]  # 32 units of [128 keys, 512 q]
                    u = 0
                    for gsz in groups:
                        s_ps = sps.tile([128, 512 * 3], F32, name=f"sps{gr}_{pair}_{u}", tag="sps")
                        for j in range(gsz):
                            kt, w = (u + j) // 2, (u + j) % 2
                            nc.tensor.matmul(
                                s_ps[:, j * 512 : (j + 1) * 512],
                                lhsT=r(ktt[w * 64 : (w + 1) * 64, kt * 128 : (kt + 1) * 128]),
                                rhs=r(qtt[w * 64 : (w + 1) * 64, :]),
                                start=True,
                                stop=True,
                            )
                        pT = ptp.tile([128, 512 * 3], BF16, name=f"pt{gr}_{pair}_{u}", tag="pt")
                        nc.scalar.activation(
                            pT[:, 0 : gsz * 512], s_ps[:, 0 : gsz * 512], EXP, scale=0.125
                        )
                        for j in range(gsz):
                            kt, w = (u + j) // 2, (u + j) % 2
                            hl = pair * 2 + w
                            nc.tensor.matmul(
                                o_ps[w * 64 : (w + 1) * 64, :],
                                lhsT=r(V_g[kt][:, hl * 64 : (hl + 1) * 64]),
                                rhs=r(pT[:, j * 512 : (j + 1) * 512]),
                                start=(kt == 0),
                                stop=(kt == 15),
                                tile_position=(0, w * 64),
                            )
                            if kt == 0:
                                nc.vector.tensor_copy(dacc[w][:], pT[:, j * 512 : (j + 1) * 512])
                            else:
                                nc.vector.tensor_add(
                                    dacc[w][:], dacc[w][:], pT[:, j * 512 : (j + 1) * 512]
                                )
                        u += gsz
                    # normalize: out_h = o_h / sum_k(exp)
                    for w in range(2):
                        red = nps.tile([128, QL], F32, name=f"red{gr}_{pair}_{w}", tag="nps")
                        nc.tensor.matmul(
                            red[0:1, :],
                            lhsT=r(ones_sb[:, 0:1]),
                            rhs=r(dacc[w][:]),
                            start=True,
                            stop=True,
                        )
                        rec = rcp.tile([1, QL], BF16, name=f"rec{gr}_{pair}_{w}", tag="rec")
                        nc.vector.reciprocal(rec[:], red[0:1, :])
                        bc = nps.tile([128, QL], F32, name=f"bc{gr}_{pair}_{w}", tag="nps")
                        nc.tensor.matmul(
                            bc[w * 64 : (w + 1) * 64, :],
                            lhsT=ones_bf[:],
                            rhs=rec[:],
                            start=True,
                            stop=True,
                            tile_position=(0, w * 64),
                        )
                        bcs = rcp.tile([128, QL], F32, name=f"bcs{gr}_{pair}_{w}", tag="bcs")
                        nc.vector.tensor_copy(
                            bcs[w * 64 : (w + 1) * 64, :], bc[w * 64 : (w + 1) * 64, :]
                        )
                        nc.vector.tensor_mul(
                            Ocat[gr * 2 + pair][w * 64 : (w + 1) * 64, :],
                            o_ps[w * 64 : (w + 1) * 64, :],
                            bcs[w * 64 : (w + 1) * 64, :],
                        )

        # ---- phase 3: output projection ----------------------------------
        with ExitStack() as ph3:
            osp = ph3.enter_context(tc.tile_pool(name="outsb", bufs=2))
            pso = ph3.enter_context(tc.tile_pool(name="pso", bufs=2, space="PSUM"))
            for qt in range(4):
                out_sb = osp.tile([128, D], F32, name=f"osb{qt}", tag="osb")
                for dc in range(2):
                    ps = pso.tile([128, 512], F32, name=f"pso{qt}_{dc}", tag="pso")
                    for kt in range(8):
                        nc.tensor.matmul(
                            ps[:],
                            lhsT=r(Ocat[kt][:, qt * 128 : (qt + 1) * 128]),
                            rhs=r(wo_sb[kt][:, dc * 512 : (dc + 1) * 512]),
                            start=(kt == 0),
                            stop=(kt == 7),
                        )
                    nc.vector.tensor_copy(out_sb[:, dc * 512 : (dc + 1) * 512], ps[:])
                nc.sync.dma_start(out_ext[qt * 128 : (qt + 1) * 128, :], out_sb[:])

    nc.compile()
    return nc


def get_nc():
    global _nc_cache
    if _nc_cache is None:
        _nc_cache = build_nc()
    return _nc_cache


def kernel(x, Wq, bq, Wk, bk, Wv, bv, Wo, bo, **extra):
    from concourse.bass_utils import run_bass_kernel_spmd

    x = np.asarray(x, dtype=np.float32)
    Wq = np.asarray(Wq, dtype=np.float32)
    Wk = np.asarray(Wk, dtype=np.float32)
    Wv = np.asarray(Wv, dtype=np.float32)
    Wo = np.asarray(Wo, dtype=np.float32)
    bq = np.asarray(bq, dtype=np.float32)
    bk = np.asarray(bk, dtype=np.float32)
    bv = np.asarray(bv, dtype=np.float32)
    bo = np.asarray(bo, dtype=np.float32)

    nc = get_nc()
    xTs = [np.ascontiguousarray(x[b].T) for b in range(B)]
    in_maps = []
    for c in range(N_CORES):
        b, g = c // 4, c % 4
        in_maps.append(
            {
                "xT": xTs[b],
                "xTq": np.ascontiguousarray(xTs[b][:, g * QL : (g + 1) * QL]),
                "wq": Wq,
                "wk": np.ascontiguousarray(Wk[:, g * GC : (g + 1) * GC]),
                "wv": np.ascontiguousarray(Wv[:, g * GC : (g + 1) * GC]),
                "wo": Wo.astype(ml_dtypes.bfloat16),
                "bq": bq,
                "bk": np.ascontiguousarray(bk[g * GC : (g + 1) * GC]),
                "bv": np.ascontiguousarray(bv[g * GC : (g + 1) * GC]),
            }
        )
    res = run_bass_kernel_spmd(nc, in_maps, core_ids=list(range(N_CORES)))
    out = np.empty((B, S, D), dtype=np.float32)
    for c in range(N_CORES):
        b, g = c // 4, c % 4
        out[b, g * QL : (g + 1) * QL, :] = res.results[c]["out"]
    out += bo
    return out


# revision 12
# speedup vs baseline: 1.0892x; 1.0892x over previous
"""Distributed multi-head attention layer for 8 TRN2 NeuronCores.

Problem: x[2,2048,1024] -> MHA(16 heads, dh=64) -> out[2,2048,1024], f32.

Sharding (per core c in 0..7):
  batch b = c//4, group g = c%4 (4 cores per batch).
  - Each core computes K/V for its 4 heads over the full sequence,
    AllGathers K/V (bf16) within its 4-core batch group,
  - computes Q for its own 512-query slice over ALL heads,
  - runs attention for all 16 heads x its 512 queries,
  - output-projects to out[b, g*512:(g+1)*512, :]. No output collective.
  Host concatenates per-batch slices and adds the output bias.

Kernel layout choices:
  - x arrives host-transposed (xT [1024,2048]) so projections need no
    on-device transpose. All matmul inputs are bf16 (host-cast weights/x);
    accumulation is f32 in PSUM.
  - Q/K are produced in [att, s] (transposed) layout; V in natural [s, dh]
    with a ones column appended per head (stride-65 layout).
  - Scores are computed transposed: S^T[k, q], so softmax's reduction axis
    (keys) is the partition axis; exp runs on ScalarE (PSUM->SBUF, scale
    folds the 1/sqrt(dh)); the ones column of V makes the AV matmul emit
    the softmax denominator as row 64 of the [65, 512] output accumulator.
    No max-subtraction is needed: scores/8 ~ N(0,1), exp is safe in f32.
  - Normalization: fast reciprocal on DVE, broadcast across partitions via
    a K=1 ones matmul, multiply on DVE. Odd heads are relocated to
    partitions 64..127 of the packed O tiles by a small SBUF->SBUF DMA.
"""

import sys

sys.path.insert(0, "/opt/trn_rl_repo")

from contextlib import ExitStack

import ml_dtypes
import numpy as np

import concourse.bass as bass
import concourse.tile as tile
from concourse import bacc, mybir

F32 = mybir.dt.float32
BF16 = mybir.dt.bfloat16
EXP = mybir.ActivationFunctionType.Exp

N_CORES = 8
B, S, D = 2, 2048, 1024
H, DH = 16, 64
QL = 512  # queries per core
GC = 256  # K/V att columns produced per core (4 heads)
VC = 4 * 65  # V tile cols: 4 heads x (64 + ones column)
KV_KT = 2 * 128 * 2048  # KT part of the kv bounce buffer (elements)
KV_V = 16 * 128 * VC  # V part (with ones columns)
KV_N = KV_KT + KV_V
RG = [[0, 1, 2, 3], [4, 5, 6, 7]]
EGROUPS = [3, 3, 3, 3, 2, 2]  # exp batching over the 16 key-tiles of a head

_nc_cache = None


def build_nc():
    nc = bacc.Bacc("TRN2", target_bir_lowering=False, debug=False, num_devices=N_CORES)

    xT_ext = nc.declare_dram_parameter("xT", [D, S], BF16, isOutput=False)
    xq_ext = nc.declare_dram_parameter("xTq", [D, QL], BF16, isOutput=False)
    wq_ext = nc.declare_dram_parameter("wq", [D, D], BF16, isOutput=False)
    wk_ext = nc.declare_dram_parameter("wk", [D, GC], BF16, isOutput=False)
    wv_ext = nc.declare_dram_parameter("wv", [D, GC], BF16, isOutput=False)
    wo_ext = nc.declare_dram_parameter("wo", [D, D], BF16, isOutput=False)
    bq_ext = nc.declare_dram_parameter("bq", [D], F32, isOutput=False)
    bk_ext = nc.declare_dram_parameter("bk", [GC], F32, isOutput=False)
    bv_ext = nc.declare_dram_parameter("bv", [GC], BF16, isOutput=False)
    out_ext = nc.declare_dram_parameter("out", [QL, D], F32, isOutput=True)

    with (
        tile.TileContext(nc) as tc,
        ExitStack() as outer,
        nc.allow_low_precision("bf16 compute; f32 PSUM accumulation"),
    ):
        # ---- long-lived pools -------------------------------------------
        cpool = outer.enter_context(tc.tile_pool(name="consts", bufs=1))
        qtpool = outer.enter_context(tc.tile_pool(name="qt", bufs=1))
        ocpool = outer.enter_context(tc.tile_pool(name="ocat", bufs=1))
        wop = outer.enter_context(tc.tile_pool(name="wo", bufs=1))
        dramp = outer.enter_context(tc.tile_pool(name="dram", bufs=1, space="DRAM"))

        ones_f32 = cpool.tile([128, 128], F32)
        nc.vector.memset(ones_f32[:], 1.0)
        ones_bf = cpool.tile([65, 128], BF16)
        nc.vector.tensor_copy(ones_bf[:], ones_f32[0:65, :])
        bq_sb = cpool.tile([128, 8], F32)
        bk_sb = cpool.tile([128, 2], F32)
        bv_sb = cpool.tile([1, GC], BF16)
        for a in range(8):
            nc.sync.dma_start(bq_sb[:, a : a + 1], bq_ext[a * 128 : (a + 1) * 128].unsqueeze(1))
        for a in range(2):
            nc.sync.dma_start(bk_sb[:, a : a + 1], bk_ext[a * 128 : (a + 1) * 128].unsqueeze(1))
        nc.sync.dma_start(bv_sb[:], bv_ext[:].unsqueeze(0))

        QT = [qtpool.tile([128, QL], BF16, name=f"qt{a}") for a in range(8)]
        Ocat = [ocpool.tile([128, QL], BF16, name=f"ocat{i}") for i in range(8)]

        kv_loc = dramp.tile([KV_N], BF16, name="kv_loc")
        kv_gath = dramp.tile([4, KV_N], BF16, name="kv_gath")
        ktl_v = kv_loc[0:KV_KT].rearrange("(t p f) -> t p f", t=2, p=128, f=2048)
        vl_v = kv_loc[KV_KT:KV_N].rearrange("(t p f) -> t p f", t=16, p=128, f=VC)

        # ---- phase 1a: K/V projection over full S ------------------------
        with ExitStack() as ph1:
            xtp = ph1.enter_context(tc.tile_pool(name="xt", bufs=1))
            wkvp = ph1.enter_context(tc.tile_pool(name="wkv", bufs=1))
            ktlp = ph1.enter_context(tc.tile_pool(name="ktloc", bufs=1))
            vlp = ph1.enter_context(tc.tile_pool(name="vloc", bufs=1))
            ps1 = ph1.enter_context(tc.tile_pool(name="ps1", bufs=3, space="PSUM"))
            ps1v = ph1.enter_context(tc.tile_pool(name="ps1v", bufs=3, space="PSUM"))

            xT = []
            for kt in range(8):
                t = xtp.tile([128, S], BF16, name=f"xt{kt}")
                nc.sync.dma_start(t[:], xT_ext[kt * 128 : (kt + 1) * 128, :])
                xT.append(t)
            wk_sb, wv_sb = [], []
            for kt in range(8):
                t = wkvp.tile([128, GC], BF16, name=f"wk{kt}")
                nc.sync.dma_start(t[:], wk_ext[kt * 128 : (kt + 1) * 128, :])
                wk_sb.append(t)
                t = wkvp.tile([128, GC], BF16, name=f"wv{kt}")
                nc.sync.dma_start(t[:], wv_ext[kt * 128 : (kt + 1) * 128, :])
                wv_sb.append(t)

            # KT_loc [256(att), 2048(s)] as 2 tiles; evict with bk bias
            KT_loc = [ktlp.tile([128, S], BF16, name=f"ktloc{a}") for a in range(2)]
            for a2 in range(2):
                for sc in range(4):
                    ps = ps1.tile([128, 512], F32, name=f"pskt{a2}_{sc}", tag="ps1")
                    for kt in range(8):
                        nc.tensor.matmul(
                            ps[:],
                            lhsT=wk_sb[kt][:, a2 * 128 : (a2 + 1) * 128],
                            rhs=xT[kt][:, sc * 512 : (sc + 1) * 512],
                            start=(kt == 0),
                            stop=(kt == 7),
                        )
                    nc.vector.tensor_scalar_add(
                        KT_loc[a2][:, sc * 512 : (sc + 1) * 512], ps[:], bk_sb[:, a2 : a2 + 1]
                    )
            # V_loc natural [2048(s), 4x(64+1)] tiles; bias via ones-matmul;
            # ones columns baked in (they travel through the AllGather)
            V_loc = [vlp.tile([128, VC], BF16, name=f"vloc{st}") for st in range(16)]
            for st in range(16):
                ps = ps1v.tile([128, GC], F32, name=f"psv{st}", tag="ps1v")
                for kt in range(8):
                    nc.tensor.matmul(
                        ps[:],
                        lhsT=xT[kt][:, st * 128 : (st + 1) * 128],
                        rhs=wv_sb[kt][:],
                        start=(kt == 0),
                        stop=False,
                    )
                nc.tensor.matmul(
                    ps[:],
                    lhsT=ones_bf[0:1, :],
                    rhs=bv_sb[:],
                    start=False,
                    stop=True,
                )
                vv = V_loc[st][:].rearrange("p (h c) -> p h c", h=4, c=65)
                nc.vector.tensor_copy(
                    vv[:, :, 0:64], ps[:].rearrange("p (h c) -> p h c", h=4, c=64)
                )
                nc.gpsimd.memset(vv[:, :, 64:65], 1.0)

            # bounce K/V to DRAM for the collective
            for a2 in range(2):
                nc.sync.dma_start(ktl_v[a2], KT_loc[a2][:])
            for st in range(16):
                nc.sync.dma_start(vl_v[st], V_loc[st][:])

        # ---- AllGather K/V within each 4-core batch group ----------------
        nc.gpsimd.collective_compute(
            "AllGather",
            mybir.AluOpType.bypass,
            replica_groups=RG,
            ins=[kv_loc.opt()],
            outs=[kv_gath.opt()],
        )

        # ---- phase 1b: Q projection (overlaps the AllGather) -------------
        with ExitStack() as ph1b:
            xqp = ph1b.enter_context(tc.tile_pool(name="xq", bufs=1))
            wqp = ph1b.enter_context(tc.tile_pool(name="wq", bufs=1))
            ps1q = ph1b.enter_context(tc.tile_pool(name="ps1q", bufs=3, space="PSUM"))

            xq_sb, wq_sb = [], []
            for kt in range(8):
                t = xqp.tile([128, QL], BF16, name=f"xq{kt}")
                nc.sync.dma_start(t[:], xq_ext[kt * 128 : (kt + 1) * 128, :])
                xq_sb.append(t)
                t = wqp.tile([128, D], BF16, name=f"wq{kt}")
                nc.sync.dma_start(t[:], wq_ext[kt * 128 : (kt + 1) * 128, :])
                wq_sb.append(t)
            for a in range(8):
                ps = ps1q.tile([128, QL], F32, name=f"psq{a}", tag="ps1q")
                for kt in range(8):
                    nc.tensor.matmul(
                        ps[:],
                        lhsT=wq_sb[kt][:, a * 128 : (a + 1) * 128],
                        rhs=xq_sb[kt][:],
                        start=(kt == 0),
                        stop=(kt == 7),
                    )
                nc.vector.tensor_scalar_add(QT[a][:], ps[:], bq_sb[:, a : a + 1])

        # ---- phase 2: attention over all 16 heads ------------------------
        with ExitStack() as ph2:
            ktgp = ph2.enter_context(tc.tile_pool(name="ktg", bufs=4))
            vgp = ph2.enter_context(tc.tile_pool(name="vg", bufs=32))
            ptp = ph2.enter_context(tc.tile_pool(name="pt", bufs=3))
            rcp = ph2.enter_context(tc.tile_pool(name="recip", bufs=2))
            sps = ph2.enter_context(tc.tile_pool(name="sps", bufs=2, space="PSUM"))
            ops = ph2.enter_context(tc.tile_pool(name="ops", bufs=1, space="PSUM"))
            nps = ph2.enter_context(tc.tile_pool(name="nps", bufs=1, space="PSUM"))

            wo_sb = []
            for kt in range(8):
                t = wop.tile([128, D], BF16, name=f"wo{kt}")
                nc.sync.dma_start(t[:], wo_ext[kt * 128 : (kt + 1) * 128, :])
                wo_sb.append(t)

            for gr in range(4):
                gv_kt = kv_gath[gr][0:KV_KT].rearrange("(t p f) -> t p f", t=2, p=128, f=2048)
                gv_v = kv_gath[gr][KV_KT:KV_N].rearrange("(t p f) -> t p f", t=16, p=128, f=VC)
                KT_g = []
                for a2 in range(2):
                    t = ktgp.tile([128, S], BF16, name=f"ktg{gr}_{a2}", tag="ktg")
                    nc.sync.dma_start(t[:], gv_kt[a2])
                    KT_g.append(t)
                V_g = []
                for st in range(16):
                    t = vgp.tile([128, VC], BF16, name=f"vg{gr}_{st}", tag="vg")
                    nc.sync.dma_start(t[:], gv_v[st])
                    V_g.append(t)

                for hl in range(4):
                    h = gr * 4 + hl
                    ktt = KT_g[hl // 2]
                    poff = (hl % 2) * 64
                    qtt = QT[h // 2]
                    o_ps = ops.tile([65, QL], F32, name=f"ops{h}", tag="ops")
                    u = 0
                    for gsz in EGROUPS:
                        s_ps = sps.tile([128, 512 * 3], F32, name=f"sps{h}_{u}", tag="sps")
                        for j in range(gsz):
                            kt = u + j
                            nc.tensor.matmul(
                                s_ps[:, j * 512 : (j + 1) * 512],
                                lhsT=ktt[poff : poff + 64, kt * 128 : (kt + 1) * 128],
                                rhs=qtt[poff : poff + 64, :],
                                start=True,
                                stop=True,
                            )
                        pT = ptp.tile([128, 512 * 3], BF16, name=f"pt{h}_{u}", tag="pt")
                        nc.scalar.activation(
                            pT[:, 0 : gsz * 512], s_ps[:, 0 : gsz * 512], EXP, scale=0.125
                        )
                        for j in range(gsz):
                            kt = u + j
                            nc.tensor.matmul(
                                o_ps[:],
                                lhsT=V_g[kt][:, hl * 65 : hl * 65 + 65],
                                rhs=pT[:, j * 512 : (j + 1) * 512],
                                start=(kt == 0),
                                stop=(kt == 15),
                            )
                        u += gsz
                    # normalize: out_h = o_h * (1 / denom), denom = row 64
                    rec_f = rcp.tile([65, QL], F32, name=f"recf{h}", tag="recf")
                    nc.vector.reciprocal(rec_f[64:65, :], o_ps[64:65, :])
                    rec_b = rcp.tile([65, QL], BF16, name=f"recb{h}", tag="recb")
                    nc.vector.tensor_copy(rec_b[64:65, :], rec_f[64:65, :])
                    bc = nps.tile([64, QL], F32, name=f"bc{h}", tag="nps")
                    nc.tensor.matmul(
                        bc[:],
                        lhsT=ones_bf[64:65, 0:64],
                        rhs=rec_b[64:65, :],
                        start=True,
                        stop=True,
                    )
                    bcs = rcp.tile([64, QL], F32, name=f"bcs{h}", tag="bcs")
                    nc.vector.tensor_copy(bcs[:], bc[:])
                    if h % 2 == 0:
                        nc.vector.tensor_mul(Ocat[h // 2][0:64, :], o_ps[0:64, :], bcs[:])
                    else:
                        osc = rcp.tile([64, QL], BF16, name=f"osc{h}", tag="osc")
                        nc.vector.tensor_mul(osc[:], o_ps[0:64, :], bcs[:])
                        nc.sync.dma_start(Ocat[h // 2][64:128, :], osc[:])

        # ---- phase 3: output projection ----------------------------------
        with ExitStack() as ph3:
            osp = ph3.enter_context(tc.tile_pool(name="outsb", bufs=2))
            pso = ph3.enter_context(tc.tile_pool(name="pso", bufs=2, space="PSUM"))
            for qt in range(4):
                out_sb = osp.tile([128, D], F32, name=f"osb{qt}", tag="osb")
                for dc in range(2):
                    ps = pso.tile([128, 512], F32, name=f"pso{qt}_{dc}", tag="pso")
                    for kt in range(8):
                        nc.tensor.matmul(
                            ps[:],
                            lhsT=Ocat[kt][:, qt * 128 : (qt + 1) * 128],
                            rhs=wo_sb[kt][:, dc * 512 : (dc + 1) * 512],
                            start=(kt == 0),
                            stop=(kt == 7),
                        )
                    nc.vector.tensor_copy(out_sb[:, dc * 512 : (dc + 1) * 512], ps[:])
                nc.sync.dma_start(out_ext[qt * 128 : (qt + 1) * 128, :], out_sb[:])

    nc.compile()
    return nc


def get_nc():
    global _nc_cache
    if _nc_cache is None:
        _nc_cache = build_nc()
    return _nc_cache


def kernel(x, Wq, bq, Wk, bk, Wv, bv, Wo, bo, **extra):
    from concourse.bass_utils import run_bass_kernel_spmd

    bf = ml_dtypes.bfloat16
    x = np.asarray(x, dtype=np.float32)
    Wq_b = np.asarray(Wq, dtype=np.float32).astype(bf)
    Wk_b = np.asarray(Wk, dtype=np.float32).astype(bf)
    Wv_b = np.asarray(Wv, dtype=np.float32).astype(bf)
    Wo_b = np.asarray(Wo, dtype=np.float32).astype(bf)
    bq = np.asarray(bq, dtype=np.float32)
    bk = np.asarray(bk, dtype=np.float32)
    bv_b = np.asarray(bv, dtype=np.float32).astype(bf)
    bo = np.asarray(bo, dtype=np.float32)

    nc = get_nc()
    xTs = [np.ascontiguousarray(x[b].T).astype(bf) for b in range(B)]
    in_maps = []
    for c in range(N_CORES):
        b, g = c // 4, c % 4
        in_maps.append(
            {
                "xT": xTs[b],
                "xTq": np.ascontiguousarray(xTs[b][:, g * QL : (g + 1) * QL]),
                "wq": Wq_b,
                "wk": np.ascontiguousarray(Wk_b[:, g * GC : (g + 1) * GC]),
                "wv": np.ascontiguousarray(Wv_b[:, g * GC : (g + 1) * GC]),
                "wo": Wo_b,
                "bq": bq,
                "bk": np.ascontiguousarray(bk[g * GC : (g + 1) * GC]),
                "bv": np.ascontiguousarray(bv_b[g * GC : (g + 1) * GC]),
            }
        )
    res = run_bass_kernel_spmd(nc, in_maps, core_ids=list(range(N_CORES)))
    out = np.empty((B, S, D), dtype=np.float32)
    for c in range(N_CORES):
        b, g = c // 4, c % 4
        out[b, g * QL : (g + 1) * QL, :] = res.results[c]["out"]
    out += bo
    return out


# revision 14
# speedup vs baseline: 1.2759x; 1.1714x over previous
"""Distributed multi-head attention layer for 8 TRN2 NeuronCores.

Problem: x[2,2048,1024] -> MHA(16 heads, dh=64) -> out[2,2048,1024], f32.

Sharding (per core c in 0..7):
  batch b = c//4, group g = c%4 (4 cores per batch).
  - Each core computes K/V for its 4 heads over the full sequence and
    AllGathers K/V (bf16) within its 4-core batch group,
  - computes Q for its own 512-query slice over ALL heads,
  - runs attention for all 16 heads x its 512 queries,
  - output-projects to out[b, g*512:(g+1)*512, :]. No output collective.
  Host concatenates per-batch slices and adds the output bias.

Overlap trick: the host permutes Wq columns / Wo rows (and bq) per core
into "local head order" (own group's 4 heads first, then groups
(g+1)%4, (g+2)%4, (g+3)%4). Attention then runs in local order: the own
4 heads read K/V straight from SBUF while the AllGather is in flight;
the 3 remote groups are read from the gathered buffer with a
partition-id-derived dynamic row index ((pid + j) % 4), keeping the
graph SPMD-identical across cores. The output projection contracts over
the permuted att axis against the identically-permuted Wo, so the
result is unchanged.

Layout choices:
  - x arrives host-transposed (xT [1024,2048]) so projections need no
    on-device transpose. All matmul inputs are bf16; PSUM accumulates f32.
  - Q/K are produced in [att, s] (transposed) layout; V in natural [s, dh]
    with a ones column per head (stride-65). Scores are computed
    transposed (S^T[k, q]) so softmax's reduction axis is the partition
    axis: exp on ScalarE (scale folds 1/sqrt(dh)); the ones column of V
    makes the AV matmul emit the softmax denominator as row 64 of the
    [65, 512] accumulator. No max-subtraction: scores/8 ~ N(0,1).
  - QK/exp/AV are software-pipelined (QK of chunk i+1 is emitted before
    AV of chunk i) so the PE never waits on ScalarE.
  - Normalization runs off the PSUM critical path: the [65,512]
    accumulator is evicted to SBUF, then fast-reciprocal -> K=1 ones
    matmul broadcast -> multiply. Odd heads are relocated to partitions
    64..127 of the packed O tiles by a small SBUF->SBUF DMA.
"""

import sys

sys.path.insert(0, "/opt/trn_rl_repo")

from contextlib import ExitStack

import ml_dtypes
import numpy as np

import concourse.bass as bass
import concourse.tile as tile
from concourse import bacc, mybir

F32 = mybir.dt.float32
BF16 = mybir.dt.bfloat16
EXP = mybir.ActivationFunctionType.Exp

N_CORES = 8
B, S, D = 2, 2048, 1024
H, DH = 16, 64
QL = 512  # queries per core
GC = 256  # K/V att columns produced per core (4 heads)
VC = 4 * 65  # V tile cols: 4 heads x (64 + ones column)
KV_KT = 2 * 128 * 2048  # KT part of the kv bounce buffer (elements)
KV_V = 16 * 128 * VC  # V part (with ones columns)
KV_N = KV_KT + KV_V
RG = [[0, 1, 2, 3], [4, 5, 6, 7]]
EGROUPS = [3, 3, 3, 3, 2, 2]  # exp batching over the 16 key-tiles of a head

_nc_cache = None


def build_nc():
    nc = bacc.Bacc("TRN2", target_bir_lowering=False, debug=False, num_devices=N_CORES)

    xT_ext = nc.declare_dram_parameter("xT", [D, S], BF16, isOutput=False)
    xq_ext = nc.declare_dram_parameter("xTq", [D, QL], BF16, isOutput=False)
    wq_ext = nc.declare_dram_parameter("wq", [D, D], BF16, isOutput=False)
    wk_ext = nc.declare_dram_parameter("wk", [D, GC], BF16, isOutput=False)
    wv_ext = nc.declare_dram_parameter("wv", [D, GC], BF16, isOutput=False)
    wo_ext = nc.declare_dram_parameter("wo", [D, D], BF16, isOutput=False)
    bq_ext = nc.declare_dram_parameter("bq", [D], F32, isOutput=False)
    bk_ext = nc.declare_dram_parameter("bk", [GC], F32, isOutput=False)
    bv_ext = nc.declare_dram_parameter("bv", [GC], BF16, isOutput=False)
    out_ext = nc.declare_dram_parameter("out", [QL, D], F32, isOutput=True)

    with (
        tile.TileContext(nc) as tc,
        ExitStack() as outer,
        nc.allow_low_precision("bf16 compute; f32 PSUM accumulation"),
    ):
        # ---- long-lived pools -------------------------------------------
        cpool = outer.enter_context(tc.tile_pool(name="consts", bufs=1))
        qtpool = outer.enter_context(tc.tile_pool(name="qt", bufs=1))
        ocpool = outer.enter_context(tc.tile_pool(name="ocat", bufs=1))
        wop = outer.enter_context(tc.tile_pool(name="wo", bufs=1))
        ktlp = outer.enter_context(tc.tile_pool(name="ktloc", bufs=1))
        vlp = outer.enter_context(tc.tile_pool(name="vloc", bufs=1))
        dramp = outer.enter_context(tc.tile_pool(name="dram", bufs=1, space="DRAM"))

        QT = [qtpool.tile([128, QL], BF16, name=f"qt{a}") for a in range(8)]
        Ocat = [ocpool.tile([128, QL], BF16, name=f"ocat{i}") for i in range(8)]
        KT_loc = [ktlp.tile([128, S], BF16, name=f"ktloc{a}") for a in range(2)]
        V_loc = [vlp.tile([128, VC], BF16, name=f"vloc{st}") for st in range(16)]

        kv_loc = dramp.tile([KV_N], BF16, name="kv_loc")
        kv_gath = dramp.tile([4, KV_N], BF16, name="kv_gath")
        ktl_v = kv_loc[0:KV_KT].rearrange("(t p f) -> t p f", t=2, p=128, f=2048)
        vl_v = kv_loc[KV_KT:KV_N].rearrange("(t p f) -> t p f", t=16, p=128, f=VC)

        # ---- phase 1a: K/V projection over full S ------------------------
        with ExitStack() as ph1:
            xtp = ph1.enter_context(tc.tile_pool(name="xt", bufs=1))
            wkvp = ph1.enter_context(tc.tile_pool(name="wkv", bufs=1))
            ps1 = ph1.enter_context(tc.tile_pool(name="ps1", bufs=3, space="PSUM"))
            ps1v = ph1.enter_context(tc.tile_pool(name="ps1v", bufs=3, space="PSUM"))

            xT = []
            for kt in range(8):
                t = xtp.tile([128, S], BF16, name=f"xt{kt}")
                nc.sync.dma_start(t[:], xT_ext[kt * 128 : (kt + 1) * 128, :])
                xT.append(t)
            wk_sb, wv_sb = [], []
            for kt in range(8):
                t = wkvp.tile([128, GC], BF16, name=f"wk{kt}")
                nc.sync.dma_start(t[:], wk_ext[kt * 128 : (kt + 1) * 128, :])
                wk_sb.append(t)
                t = wkvp.tile([128, GC], BF16, name=f"wv{kt}")
                nc.sync.dma_start(t[:], wv_ext[kt * 128 : (kt + 1) * 128, :])
                wv_sb.append(t)
            # small constants on the gpsimd DMA queue, off the critical path
            ones_f32 = cpool.tile([128, 128], F32)
            nc.vector.memset(ones_f32[:], 1.0)
            ones_bf = cpool.tile([65, 128], BF16)
            nc.vector.tensor_copy(ones_bf[:], ones_f32[0:65, :])
            bq_sb = cpool.tile([128, 8], F32)
            bk_sb = cpool.tile([128, 2], F32)
            bv_sb = cpool.tile([1, GC], BF16)
            for a in range(8):
                nc.gpsimd.dma_start(
                    bq_sb[:, a : a + 1], bq_ext[a * 128 : (a + 1) * 128].unsqueeze(1)
                )
            for a in range(2):
                nc.gpsimd.dma_start(
                    bk_sb[:, a : a + 1], bk_ext[a * 128 : (a + 1) * 128].unsqueeze(1)
                )
            nc.gpsimd.dma_start(bv_sb[:], bv_ext[:].unsqueeze(0))

            # KT_loc [256(att), 2048(s)] as 2 tiles; evict with bk bias
            for a2 in range(2):
                for sc in range(4):
                    ps = ps1.tile([128, 512], F32, name=f"pskt{a2}_{sc}", tag="ps1")
                    for kt in range(8):
                        nc.tensor.matmul(
                            ps[:],
                            lhsT=wk_sb[kt][:, a2 * 128 : (a2 + 1) * 128],
                            rhs=xT[kt][:, sc * 512 : (sc + 1) * 512],
                            start=(kt == 0),
                            stop=(kt == 7),
                        )
                    nc.vector.tensor_scalar_add(
                        KT_loc[a2][:, sc * 512 : (sc + 1) * 512], ps[:], bk_sb[:, a2 : a2 + 1]
                    )
                nc.sync.dma_start(ktl_v[a2], KT_loc[a2][:])
            # V_loc natural [2048(s), 4x(64+1)] tiles; bias via ones-matmul;
            # ones columns baked in (they travel through the AllGather)
            for st in range(16):
                ps = ps1v.tile([128, GC], F32, name=f"psv{st}", tag="ps1v")
                for kt in range(8):
                    nc.tensor.matmul(
                        ps[:],
                        lhsT=xT[kt][:, st * 128 : (st + 1) * 128],
                        rhs=wv_sb[kt][:],
                        start=(kt == 0),
                        stop=False,
                    )
                nc.tensor.matmul(
                    ps[:], lhsT=ones_bf[0:1, :], rhs=bv_sb[:], start=False, stop=True
                )
                vv = V_loc[st][:].rearrange("p (h c) -> p h c", h=4, c=65)
                nc.vector.tensor_copy(
                    vv[:, :, 0:64], ps[:].rearrange("p (h c) -> p h c", h=4, c=64)
                )
                nc.gpsimd.memset(vv[:, :, 64:65], 1.0)
                nc.sync.dma_start(vl_v[st], V_loc[st][:])

        # ---- AllGather K/V within each 4-core batch group ----------------
        nc.gpsimd.collective_compute(
            "AllGather",
            mybir.AluOpType.bypass,
            replica_groups=RG,
            ins=[kv_loc.opt()],
            outs=[kv_gath.opt()],
        )

        # ---- phase 1b: Q projection (overlaps the AllGather) -------------
        with ExitStack() as ph1b:
            xqp = ph1b.enter_context(tc.tile_pool(name="xq", bufs=1))
            wqp = ph1b.enter_context(tc.tile_pool(name="wq", bufs=1))
            ps1q = ph1b.enter_context(tc.tile_pool(name="ps1q", bufs=3, space="PSUM"))

            xq_sb, wq_sb = [], []
            for kt in range(8):
                t = xqp.tile([128, QL], BF16, name=f"xq{kt}")
                nc.sync.dma_start(t[:], xq_ext[kt * 128 : (kt + 1) * 128, :])
                xq_sb.append(t)
                t = wqp.tile([128, D], BF16, name=f"wq{kt}")
                nc.sync.dma_start(t[:], wq_ext[kt * 128 : (kt + 1) * 128, :])
                wq_sb.append(t)
            for a in range(8):
                ps = ps1q.tile([128, QL], F32, name=f"psq{a}", tag="ps1q")
                for kt in range(8):
                    nc.tensor.matmul(
                        ps[:],
                        lhsT=wq_sb[kt][:, a * 128 : (a + 1) * 128],
                        rhs=xq_sb[kt][:],
                        start=(kt == 0),
                        stop=(kt == 7),
                    )
                nc.vector.tensor_scalar_add(QT[a][:], ps[:], bq_sb[:, a : a + 1])

        # ---- phase 2: attention, local head order ------------------------
        with ExitStack() as ph2:
            ktgp = ph2.enter_context(tc.tile_pool(name="ktg", bufs=4))
            vgp = ph2.enter_context(tc.tile_pool(name="vg", bufs=32))
            ptp = ph2.enter_context(tc.tile_pool(name="pt", bufs=3))
            rcp = ph2.enter_context(tc.tile_pool(name="recip", bufs=2))
            sps = ph2.enter_context(tc.tile_pool(name="sps", bufs=2, space="PSUM"))
            ops = ph2.enter_context(tc.tile_pool(name="ops", bufs=1, space="PSUM"))
            nps = ph2.enter_context(tc.tile_pool(name="nps", bufs=1, space="PSUM"))

            wo_sb = []
            for kt in range(8):
                t = wop.tile([128, D], BF16, name=f"wo{kt}")
                nc.sync.dma_start(t[:], wo_ext[kt * 128 : (kt + 1) * 128, :])
                wo_sb.append(t)

            # dynamic rows for the 3 remote groups: (pid + j) % 4
            pid = nc.sync.partition_id()
            row_vals = []
            for j in (1, 2, 3):
                rj = nc.sync.alloc_register(f"kvrow{j}")
                nc.sync.reg_alu(rj, pid, j, mybir.AluOpType.add)
                nc.sync.reg_alu(rj, rj, 4, mybir.AluOpType.mod)
                row_vals.append(nc.sync.snap(rj, donate=True, min_val=0, max_val=3))

            def attend_head(lg, hl, ktt, poff, V_tiles):
                """One head: 16 key-tiles, software-pipelined QK/exp/AV."""
                lh = lg * 4 + hl  # local head index
                qtt = QT[lh // 2]
                qoff = (lh % 2) * 64
                o_ps = ops.tile([65, QL], F32, name=f"ops{lh}", tag="ops")
                pend = []
                u = 0

                def flush(ent):
                    s_ps, u0, gsz = ent
                    pT = ptp.tile([128, 512 * 3], BF16, name=f"pt{lh}_{u0}", tag="pt")
                    nc.scalar.activation(
                        pT[:, 0 : gsz * 512], s_ps[:, 0 : gsz * 512], EXP, scale=0.125
                    )
                    for j in range(gsz):
                        kt = u0 + j
                        nc.tensor.matmul(
                            o_ps[:],
                            lhsT=V_tiles[kt][:, hl * 65 : hl * 65 + 65],
                            rhs=pT[:, j * 512 : (j + 1) * 512],
                            start=(kt == 0),
                            stop=(kt == 15),
                        )

                for gsz in EGROUPS:
                    s_ps = sps.tile([128, 512 * 3], F32, name=f"sps{lh}_{u}", tag="sps")
                    for j in range(gsz):
                        kt = u + j
                        nc.tensor.matmul(
                            s_ps[:, j * 512 : (j + 1) * 512],
                            lhsT=ktt[poff : poff + 64, kt * 128 : (kt + 1) * 128],
                            rhs=qtt[qoff : qoff + 64, :],
                            start=True,
                            stop=True,
                        )
                    pend.append((s_ps, u, gsz))
                    u += gsz
                    if len(pend) == 2:
                        flush(pend.pop(0))
                while pend:
                    flush(pend.pop(0))

                # normalization, off the PSUM critical path
                o_sb = rcp.tile([65, QL], F32, name=f"osb{lh}", tag="osb65")
                nc.vector.tensor_copy(o_sb[:], o_ps[:])
                rec_f = rcp.tile([65, QL], F32, name=f"recf{lh}", tag="recf")
                nc.vector.reciprocal(rec_f[64:65, :], o_sb[64:65, :])
                rec_b = rcp.tile([65, QL], BF16, name=f"recb{lh}", tag="recb")
                nc.vector.tensor_copy(rec_b[64:65, :], rec_f[64:65, :])
                bc = nps.tile([64, QL], F32, name=f"bc{lh}", tag="nps")
                nc.tensor.matmul(
                    bc[:],
                    lhsT=ones_bf[64:65, 0:64],
                    rhs=rec_b[64:65, :],
                    start=True,
                    stop=True,
                )
                bcs = rcp.tile([64, QL], F32, name=f"bcs{lh}", tag="bcs")
                nc.vector.tensor_copy(bcs[:], bc[:])
                if lh % 2 == 0:
                    nc.vector.tensor_mul(Ocat[lh // 2][0:64, :], o_sb[0:64, :], bcs[:])
                else:
                    osc = rcp.tile([64, QL], BF16, name=f"osc{lh}", tag="osc")
                    nc.vector.tensor_mul(osc[:], o_sb[0:64, :], bcs[:])
                    nc.sync.dma_start(Ocat[lh // 2][64:128, :], osc[:])

            # local group 0: own K/V straight from SBUF (no AllGather wait)
            for hl in range(4):
                attend_head(0, hl, KT_loc[hl // 2], (hl % 2) * 64, V_loc)

            # local groups 1..3: gathered K/V at dynamic row (pid + j) % 4
            for lg in (1, 2, 3):
                grow = kv_gath[bass.ds(row_vals[lg - 1], 1)]
                gv_kt = grow[:, 0:KV_KT].rearrange(
                    "o (t p f) -> o t p f", t=2, p=128, f=2048
                )
                gv_v = grow[:, KV_KT:KV_N].rearrange(
                    "o (t p f) -> o t p f", t=16, p=128, f=VC
                )
                KT_g = []
                for a2 in range(2):
                    t = ktgp.tile([128, S], BF16, name=f"ktg{lg}_{a2}", tag="ktg")
                    nc.sync.dma_start(t[:], gv_kt[0, a2])
                    KT_g.append(t)
                V_g = []
                for st in range(16):
                    t = vgp.tile([128, VC], BF16, name=f"vg{lg}_{st}", tag="vg")
                    nc.sync.dma_start(t[:], gv_v[0, st])
                    V_g.append(t)
                for hl in range(4):
                    attend_head(lg, hl, KT_g[hl // 2], (hl % 2) * 64, V_g)

        # ---- phase 3: output projection (permuted att axis) --------------
        with ExitStack() as ph3:
            osp = ph3.enter_context(tc.tile_pool(name="outsb", bufs=2))
            pso = ph3.enter_context(tc.tile_pool(name="pso", bufs=2, space="PSUM"))
            for qt in range(4):
                out_sb = osp.tile([128, D], F32, name=f"osb{qt}", tag="osb")
                for dc in range(2):
                    ps = pso.tile([128, 512], F32, name=f"pso{qt}_{dc}", tag="pso")
                    for kt in range(8):
                        nc.tensor.matmul(
                            ps[:],
                            lhsT=Ocat[kt][:, qt * 128 : (qt + 1) * 128],
                            rhs=wo_sb[kt][:, dc * 512 : (dc + 1) * 512],
                            start=(kt == 0),
                            stop=(kt == 7),
                        )
                    nc.vector.tensor_copy(out_sb[:, dc * 512 : (dc + 1) * 512], ps[:])
                nc.sync.dma_start(out_ext[qt * 128 : (qt + 1) * 128, :], out_sb[:])

    nc.compile()
    return nc


def get_nc():
    global _nc_cache
    if _nc_cache is None:
        _nc_cache = build_nc()
    return _nc_cache


def kernel(x, Wq, bq, Wk, bk, Wv, bv, Wo, bo, **extra):
    from concourse.bass_utils import run_bass_kernel_spmd

    bf = ml_dtypes.bfloat16
    x = np.asarray(x, dtype=np.float32)
    Wq_b = np.asarray(Wq, dtype=np.float32).astype(bf)
    Wk_b = np.asarray(Wk, dtype=np.float32).astype(bf)
    Wv_b = np.asarray(Wv, dtype=np.float32).astype(bf)
    Wo_b = np.asarray(Wo, dtype=np.float32).astype(bf)
    bq = np.asarray(bq, dtype=np.float32)
    bk = np.asarray(bk, dtype=np.float32)
    bv_b = np.asarray(bv, dtype=np.float32).astype(bf)
    bo = np.asarray(bo, dtype=np.float32)

    nc = get_nc()
    xTs = [np.ascontiguousarray(x[b].T).astype(bf) for b in range(B)]
    in_maps = []
    for c in range(N_CORES):
        b, g = c // 4, c % 4
        # local head order: att columns of group (g+j)%4 come j-th
        perm = np.concatenate(
            [np.arange(((g + j) % 4) * GC, ((g + j) % 4 + 1) * GC) for j in range(4)]
        )
        in_maps.append(
            {
                "xT": xTs[b],
                "xTq": np.ascontiguousarray(xTs[b][:, g * QL : (g + 1) * QL]),
                "wq": np.ascontiguousarray(Wq_b[:, perm]),
                "wk": np.ascontiguousarray(Wk_b[:, g * GC : (g + 1) * GC]),
                "wv": np.ascontiguousarray(Wv_b[:, g * GC : (g + 1) * GC]),
                "wo": np.ascontiguousarray(Wo_b[perm, :]),
                "bq": np.ascontiguousarray(bq[perm]),
                "bk": np.ascontiguousarray(bk[g * GC : (g + 1) * GC]),
                "bv": np.ascontiguousarray(bv_b[g * GC : (g + 1) * GC]),
            }
        )
    res = run_bass_kernel_spmd(nc, in_maps, core_ids=list(range(N_CORES)))
    out = np.empty((B, S, D), dtype=np.float32)
    for c in range(N_CORES):
        b, g = c // 4, c % 4
        out[b, g * QL : (g + 1) * QL, :] = res.results[c]["out"]
    out += bo
    return out


# revision 15
# speedup vs baseline: 1.3181x; 1.0331x over previous
"""Distributed multi-head attention layer for 8 TRN2 NeuronCores.

Problem: x[2,2048,1024] -> MHA(16 heads, dh=64) -> out[2,2048,1024], f32.

Sharding (per core c in 0..7):
  batch b = c//4, group g = c%4 (4 cores per batch).
  - Each core computes K/V for its 4 heads over the full sequence and
    AllGathers K/V (bf16) within its 4-core batch group,
  - computes Q for its own 512-query slice over ALL heads,
  - runs attention for all 16 heads x its 512 queries,
  - output-projects to out[b, g*512:(g+1)*512, :]. No output collective.
  Host concatenates per-batch slices and adds the output bias.

Overlap trick: the host permutes Wq columns / Wo rows (and bq) per core
into "local head order" (own group's 4 heads first, then groups
(g+1)%4, (g+2)%4, (g+3)%4). Attention then runs in local order: the own
4 heads read K/V straight from SBUF while the AllGather is in flight;
the 3 remote groups are read from the gathered buffer with a
partition-id-derived dynamic row index ((pid + j) % 4), keeping the
graph SPMD-identical across cores. The output projection contracts over
the permuted att axis against the identically-permuted Wo, so the
result is unchanged.

Layout choices:
  - x arrives host-transposed (xT [1024,2048]) so projections need no
    on-device transpose. All matmul inputs are bf16; PSUM accumulates f32.
  - Q/K are produced in [att, s] (transposed) layout; V in natural [s, dh]
    with a ones column per head (stride-65). Scores are computed
    transposed (S^T[k, q]) so softmax's reduction axis is the partition
    axis: exp on ScalarE (scale folds 1/sqrt(dh)); the ones column of V
    makes the AV matmul emit the softmax denominator as row 64 of the
    [65, 512] accumulator. No max-subtraction: scores/8 ~ N(0,1).
  - QK/exp/AV are software-pipelined (QK of chunk i+1 is emitted before
    AV of chunk i) so the PE never waits on ScalarE.
  - Normalization runs off the PSUM critical path: the [65,512]
    accumulator is evicted to SBUF, then fast-reciprocal -> K=1 ones
    matmul broadcast -> multiply. Odd heads are relocated to partitions
    64..127 of the packed O tiles by a small SBUF->SBUF DMA.
"""

import sys

sys.path.insert(0, "/opt/trn_rl_repo")

from contextlib import ExitStack

import ml_dtypes
import numpy as np

import concourse.bass as bass
import concourse.tile as tile
from concourse import bacc, mybir

F32 = mybir.dt.float32
BF16 = mybir.dt.bfloat16
EXP = mybir.ActivationFunctionType.Exp

N_CORES = 8
B, S, D = 2, 2048, 1024
H, DH = 16, 64
QL = 512  # queries per core
GC = 256  # K/V att columns produced per core (4 heads)
VC = 4 * 65  # V tile cols: 4 heads x (64 + ones column)
KV_KT = 2 * 128 * 2048  # KT part of the kv bounce buffer (elements)
KV_V = 16 * 128 * VC  # V part (with ones columns)
KV_N = KV_KT + KV_V
RG = [[0, 1, 2, 3], [4, 5, 6, 7]]
EGROUPS = [2] * 8  # exp batching over the 16 key-tiles of a head

_nc_cache = None


def build_nc():
    nc = bacc.Bacc("TRN2", target_bir_lowering=False, debug=False, num_devices=N_CORES)

    xT_ext = nc.declare_dram_parameter("xT", [D, S], BF16, isOutput=False)
    xq_ext = nc.declare_dram_parameter("xTq", [D, QL], BF16, isOutput=False)
    wq_ext = nc.declare_dram_parameter("wq", [D, D], BF16, isOutput=False)
    wk_ext = nc.declare_dram_parameter("wk", [D, GC], BF16, isOutput=False)
    wv_ext = nc.declare_dram_parameter("wv", [D, GC], BF16, isOutput=False)
    wo_ext = nc.declare_dram_parameter("wo", [D, D], BF16, isOutput=False)
    bq_ext = nc.declare_dram_parameter("bq", [D], F32, isOutput=False)
    bk_ext = nc.declare_dram_parameter("bk", [GC], F32, isOutput=False)
    bv_ext = nc.declare_dram_parameter("bv", [GC], BF16, isOutput=False)
    out_ext = nc.declare_dram_parameter("out", [QL, D], F32, isOutput=True)

    with (
        tile.TileContext(nc) as tc,
        ExitStack() as outer,
        nc.allow_low_precision("bf16 compute; f32 PSUM accumulation"),
    ):
        # ---- long-lived pools -------------------------------------------
        cpool = outer.enter_context(tc.tile_pool(name="consts", bufs=1))
        qtpool = outer.enter_context(tc.tile_pool(name="qt", bufs=1))
        ocpool = outer.enter_context(tc.tile_pool(name="ocat", bufs=1))
        wop = outer.enter_context(tc.tile_pool(name="wo", bufs=1))
        ktlp = outer.enter_context(tc.tile_pool(name="ktloc", bufs=1))
        vlp = outer.enter_context(tc.tile_pool(name="vloc", bufs=1))
        dramp = outer.enter_context(tc.tile_pool(name="dram", bufs=1, space="DRAM"))

        QT = [qtpool.tile([128, QL], BF16, name=f"qt{a}") for a in range(8)]
        Ocat = [ocpool.tile([128, QL], BF16, name=f"ocat{i}") for i in range(8)]
        KT_loc = [ktlp.tile([128, S], BF16, name=f"ktloc{a}") for a in range(2)]
        V_loc = [vlp.tile([128, VC], BF16, name=f"vloc{st}") for st in range(16)]

        kv_loc = dramp.tile([KV_N], BF16, name="kv_loc")
        kv_gath = dramp.tile([4, KV_N], BF16, name="kv_gath")
        ktl_v = kv_loc[0:KV_KT].rearrange("(t p f) -> t p f", t=2, p=128, f=2048)
        vl_v = kv_loc[KV_KT:KV_N].rearrange("(t p f) -> t p f", t=16, p=128, f=VC)

        # ---- phase 1a: K/V projection over full S ------------------------
        with ExitStack() as ph1:
            xtp = ph1.enter_context(tc.tile_pool(name="xt", bufs=1))
            wkvp = ph1.enter_context(tc.tile_pool(name="wkv", bufs=1))
            ps1 = ph1.enter_context(tc.tile_pool(name="ps1", bufs=3, space="PSUM"))
            ps1v = ph1.enter_context(tc.tile_pool(name="ps1v", bufs=3, space="PSUM"))

            xT = []
            for kt in range(8):
                t = xtp.tile([128, S], BF16, name=f"xt{kt}")
                nc.sync.dma_start(t[:], xT_ext[kt * 128 : (kt + 1) * 128, :])
                xT.append(t)
            wk_sb, wv_sb = [], []
            for kt in range(8):
                t = wkvp.tile([128, GC], BF16, name=f"wk{kt}")
                nc.sync.dma_start(t[:], wk_ext[kt * 128 : (kt + 1) * 128, :])
                wk_sb.append(t)
                t = wkvp.tile([128, GC], BF16, name=f"wv{kt}")
                nc.sync.dma_start(t[:], wv_ext[kt * 128 : (kt + 1) * 128, :])
                wv_sb.append(t)
            # small constants on the gpsimd DMA queue, off the critical path
            ones_f32 = cpool.tile([128, 128], F32)
            nc.vector.memset(ones_f32[:], 1.0)
            ones_bf = cpool.tile([65, 128], BF16)
            nc.vector.tensor_copy(ones_bf[:], ones_f32[0:65, :])
            bq_sb = cpool.tile([128, 8], F32)
            bk_sb = cpool.tile([128, 2], F32)
            bv_sb = cpool.tile([1, GC], BF16)
            for a in range(8):
                nc.gpsimd.dma_start(
                    bq_sb[:, a : a + 1], bq_ext[a * 128 : (a + 1) * 128].unsqueeze(1)
                )
            for a in range(2):
                nc.gpsimd.dma_start(
                    bk_sb[:, a : a + 1], bk_ext[a * 128 : (a + 1) * 128].unsqueeze(1)
                )
            nc.gpsimd.dma_start(bv_sb[:], bv_ext[:].unsqueeze(0))

            # KT_loc [256(att), 2048(s)] as 2 tiles; evict with bk bias
            for a2 in range(2):
                for sc in range(4):
                    ps = ps1.tile([128, 512], F32, name=f"pskt{a2}_{sc}", tag="ps1")
                    for kt in range(8):
                        nc.tensor.matmul(
                            ps[:],
                            lhsT=wk_sb[kt][:, a2 * 128 : (a2 + 1) * 128],
                            rhs=xT[kt][:, sc * 512 : (sc + 1) * 512],
                            start=(kt == 0),
                            stop=(kt == 7),
                        )
                    nc.vector.tensor_scalar_add(
                        KT_loc[a2][:, sc * 512 : (sc + 1) * 512], ps[:], bk_sb[:, a2 : a2 + 1]
                    )
                nc.sync.dma_start(ktl_v[a2], KT_loc[a2][:])
            # V_loc natural [2048(s), 4x(64+1)] tiles; bias via ones-matmul;
            # ones columns baked in (they travel through the AllGather)
            for st in range(16):
                ps = ps1v.tile([128, GC], F32, name=f"psv{st}", tag="ps1v")
                for kt in range(8):
                    nc.tensor.matmul(
                        ps[:],
                        lhsT=xT[kt][:, st * 128 : (st + 1) * 128],
                        rhs=wv_sb[kt][:],
                        start=(kt == 0),
                        stop=False,
                    )
                nc.tensor.matmul(
                    ps[:], lhsT=ones_bf[0:1, :], rhs=bv_sb[:], start=False, stop=True
                )
                vv = V_loc[st][:].rearrange("p (h c) -> p h c", h=4, c=65)
                nc.vector.tensor_copy(
                    vv[:, :, 0:64], ps[:].rearrange("p (h c) -> p h c", h=4, c=64)
                )
                nc.gpsimd.memset(vv[:, :, 64:65], 1.0)
                nc.sync.dma_start(vl_v[st], V_loc[st][:])

        # ---- AllGather K/V within each 4-core batch group ----------------
        nc.gpsimd.collective_compute(
            "AllGather",
            mybir.AluOpType.bypass,
            replica_groups=RG,
            ins=[kv_loc.opt()],
            outs=[kv_gath.opt()],
        )

        # ---- phase 1b: Q projection (overlaps the AllGather) -------------
        with ExitStack() as ph1b:
            xqp = ph1b.enter_context(tc.tile_pool(name="xq", bufs=1))
            wqp = ph1b.enter_context(tc.tile_pool(name="wq", bufs=1))
            ps1q = ph1b.enter_context(tc.tile_pool(name="ps1q", bufs=3, space="PSUM"))

            xq_sb, wq_sb = [], []
            for kt in range(8):
                t = xqp.tile([128, QL], BF16, name=f"xq{kt}")
                nc.sync.dma_start(t[:], xq_ext[kt * 128 : (kt + 1) * 128, :])
                xq_sb.append(t)
                t = wqp.tile([128, D], BF16, name=f"wq{kt}")
                nc.sync.dma_start(t[:], wq_ext[kt * 128 : (kt + 1) * 128, :])
                wq_sb.append(t)
            for a in range(8):
                ps = ps1q.tile([128, QL], F32, name=f"psq{a}", tag="ps1q")
                for kt in range(8):
                    nc.tensor.matmul(
                        ps[:],
                        lhsT=wq_sb[kt][:, a * 128 : (a + 1) * 128],
                        rhs=xq_sb[kt][:],
                        start=(kt == 0),
                        stop=(kt == 7),
                    )
                nc.vector.tensor_scalar_add(QT[a][:], ps[:], bq_sb[:, a : a + 1])

        # ---- phase 2: attention, local head order ------------------------
        with ExitStack() as ph2:
            ktgp = ph2.enter_context(tc.tile_pool(name="ktg", bufs=4))
            vgp = ph2.enter_context(tc.tile_pool(name="vg", bufs=32))
            ptp = ph2.enter_context(tc.tile_pool(name="pt", bufs=3))
            rcp = ph2.enter_context(tc.tile_pool(name="recip", bufs=2))
            sps = ph2.enter_context(tc.tile_pool(name="sps", bufs=2, space="PSUM"))
            ops = ph2.enter_context(tc.tile_pool(name="ops", bufs=4, space="PSUM"))

            wo_sb = []
            for kt in range(8):
                t = wop.tile([128, D], BF16, name=f"wo{kt}")
                nc.sync.dma_start(t[:], wo_ext[kt * 128 : (kt + 1) * 128, :])
                wo_sb.append(t)

            # dynamic rows for the 3 remote groups: (pid + j) % 4
            pid = nc.sync.partition_id()
            row_vals = []
            for j in (1, 2, 3):
                rj = nc.sync.alloc_register(f"kvrow{j}")
                nc.sync.reg_alu(rj, pid, j, mybir.AluOpType.add)
                nc.sync.reg_alu(rj, rj, 4, mybir.AluOpType.mod)
                row_vals.append(nc.sync.snap(rj, donate=True, min_val=0, max_val=3))

            def attend_head(lg, hl, ktt, poff, V_tiles):
                """One head: 16 key-tiles, software-pipelined QK/exp/AV."""
                lh = lg * 4 + hl  # local head index
                qtt = QT[lh // 2]
                qoff = (lh % 2) * 64
                o_ps = ops.tile([65, QL], F32, name=f"ops{lh}", tag="ops")
                pend = []
                u = 0

                def flush(ent):
                    s_ps, u0, gsz = ent
                    pT = ptp.tile([128, 512 * 2], BF16, name=f"pt{lh}_{u0}", tag="pt")
                    nc.scalar.activation(
                        pT[:, 0 : gsz * 512], s_ps[:, 0 : gsz * 512], EXP, scale=0.125
                    )
                    for j in range(gsz):
                        kt = u0 + j
                        nc.tensor.matmul(
                            o_ps[:],
                            lhsT=V_tiles[kt][:, hl * 65 : hl * 65 + 65],
                            rhs=pT[:, j * 512 : (j + 1) * 512],
                            start=(kt == 0),
                            stop=(kt == 15),
                        )

                for gsz in EGROUPS:
                    s_ps = sps.tile([128, 512 * 2], F32, name=f"sps{lh}_{u}", tag="sps")
                    for j in range(gsz):
                        kt = u + j
                        nc.tensor.matmul(
                            s_ps[:, j * 512 : (j + 1) * 512],
                            lhsT=ktt[poff : poff + 64, kt * 128 : (kt + 1) * 128],
                            rhs=qtt[qoff : qoff + 64, :],
                            start=True,
                            stop=True,
                        )
                    pend.append((s_ps, u, gsz))
                    u += gsz
                    if len(pend) == 2:
                        flush(pend.pop(0))
                while pend:
                    flush(pend.pop(0))

                # normalization, off the PSUM critical path
                o_sb = rcp.tile([65, QL], F32, name=f"osb{lh}", tag="osb65")
                nc.vector.tensor_copy(o_sb[:], o_ps[:])
                rec_f = rcp.tile([65, QL], F32, name=f"recf{lh}", tag="recf")
                nc.vector.reciprocal(rec_f[64:65, :], o_sb[64:65, :])
                rec_b = rcp.tile([65, QL], BF16, name=f"recb{lh}", tag="recb")
                nc.vector.tensor_copy(rec_b[64:65, :], rec_f[64:65, :])
                bc = ops.tile([65, QL], F32, name=f"bc{lh}", tag="ops")
                nc.tensor.matmul(
                    bc[0:64, :],
                    lhsT=ones_bf[64:65, 0:64],
                    rhs=rec_b[64:65, :],
                    start=True,
                    stop=True,
                )
                bcs = rcp.tile([64, QL], F32, name=f"bcs{lh}", tag="bcs")
                nc.vector.tensor_copy(bcs[:], bc[0:64, :])
                if lh % 2 == 0:
                    nc.vector.tensor_mul(Ocat[lh // 2][0:64, :], o_sb[0:64, :], bcs[:])
                else:
                    osc = rcp.tile([64, QL], BF16, name=f"osc{lh}", tag="osc")
                    nc.vector.tensor_mul(osc[:], o_sb[0:64, :], bcs[:])
                    nc.sync.dma_start(Ocat[lh // 2][64:128, :], osc[:])

            # local group 0: own K/V straight from SBUF (no AllGather wait)
            for hl in range(4):
                attend_head(0, hl, KT_loc[hl // 2], (hl % 2) * 64, V_loc)

            # local groups 1..3: gathered K/V at dynamic row (pid + j) % 4
            for lg in (1, 2, 3):
                grow = kv_gath[bass.ds(row_vals[lg - 1], 1)]
                gv_kt = grow[:, 0:KV_KT].rearrange(
                    "o (t p f) -> o t p f", t=2, p=128, f=2048
                )
                gv_v = grow[:, KV_KT:KV_N].rearrange(
                    "o (t p f) -> o t p f", t=16, p=128, f=VC
                )
                KT_g = []
                for a2 in range(2):
                    t = ktgp.tile([128, S], BF16, name=f"ktg{lg}_{a2}", tag="ktg")
                    nc.sync.dma_start(t[:], gv_kt[0, a2])
                    KT_g.append(t)
                V_g = []
                for st in range(16):
                    t = vgp.tile([128, VC], BF16, name=f"vg{lg}_{st}", tag="vg")
                    nc.sync.dma_start(t[:], gv_v[0, st])
                    V_g.append(t)
                for hl in range(4):
                    attend_head(lg, hl, KT_g[hl // 2], (hl % 2) * 64, V_g)

        # ---- phase 3: output projection (permuted att axis) --------------
        with ExitStack() as ph3:
            osp = ph3.enter_context(tc.tile_pool(name="outsb", bufs=2))
            pso = ph3.enter_context(tc.tile_pool(name="pso", bufs=2, space="PSUM"))
            for qt in range(4):
                out_sb = osp.tile([128, D], F32, name=f"osb{qt}", tag="osb")
                for dc in range(2):
                    ps = pso.tile([128, 512], F32, name=f"pso{qt}_{dc}", tag="pso")
                    for kt in range(8):
                        nc.tensor.matmul(
                            ps[:],
                            lhsT=Ocat[kt][:, qt * 128 : (qt + 1) * 128],
                            rhs=wo_sb[kt][:, dc * 512 : (dc + 1) * 512],
                            start=(kt == 0),
                            stop=(kt == 7),
                        )
                    nc.vector.tensor_copy(out_sb[:, dc * 512 : (dc + 1) * 512], ps[:])
                nc.sync.dma_start(out_ext[qt * 128 : (qt + 1) * 128, :], out_sb[:])

    nc.compile()
    return nc


def get_nc():
    global _nc_cache
    if _nc_cache is None:
        _nc_cache = build_nc()
    return _nc_cache


def kernel(x, Wq, bq, Wk, bk, Wv, bv, Wo, bo, **extra):
    from concourse.bass_utils import run_bass_kernel_spmd

    bf = ml_dtypes.bfloat16
    x = np.asarray(x, dtype=np.float32)
    Wq_b = np.asarray(Wq, dtype=np.float32).astype(bf)
    Wk_b = np.asarray(Wk, dtype=np.float32).astype(bf)
    Wv_b = np.asarray(Wv, dtype=np.float32).astype(bf)
    Wo_b = np.asarray(Wo, dtype=np.float32).astype(bf)
    bq = np.asarray(bq, dtype=np.float32)
    bk = np.asarray(bk, dtype=np.float32)
    bv_b = np.asarray(bv, dtype=np.float32).astype(bf)
    bo = np.asarray(bo, dtype=np.float32)

    nc = get_nc()
    xTs = [np.ascontiguousarray(x[b].T).astype(bf) for b in range(B)]
    in_maps = []
    for c in range(N_CORES):
        b, g = c // 4, c % 4
        # local head order: att columns of group (g+j)%4 come j-th
        perm = np.concatenate(
            [np.arange(((g + j) % 4) * GC, ((g + j) % 4 + 1) * GC) for j in range(4)]
        )
        in_maps.append(
            {
                "xT": xTs[b],
                "xTq": np.ascontiguousarray(xTs[b][:, g * QL : (g + 1) * QL]),
                "wq": np.ascontiguousarray(Wq_b[:, perm]),
                "wk": np.ascontiguousarray(Wk_b[:, g * GC : (g + 1) * GC]),
                "wv": np.ascontiguousarray(Wv_b[:, g * GC : (g + 1) * GC]),
                "wo": np.ascontiguousarray(Wo_b[perm, :]),
                "bq": np.ascontiguousarray(bq[perm]),
                "bk": np.ascontiguousarray(bk[g * GC : (g + 1) * GC]),
                "bv": np.ascontiguousarray(bv_b[g * GC : (g + 1) * GC]),
            }
        )
    res = run_bass_kernel_spmd(nc, in_maps, core_ids=list(range(N_CORES)))
    out = np.empty((B, S, D), dtype=np.float32)
    for c in range(N_CORES):
        b, g = c // 4, c % 4
        out[b, g * QL : (g + 1) * QL, :] = res.results[c]["out"]
    out += bo
    return out
